# revision 4
# baseline (speedup 1.0000x reference)
"""Periodic-boundary fixed-capacity neighbour list on 8 trn2 NeuronCores.

Algorithm (device, per core, rows sharded 256/core as 2 partition-tiles):
  For unit cell + cutoff 0.3, a pair (i, j) can be within cutoff for at most
  ONE of the 27 periodic images (per-axis shift intervals are disjoint), so
  the N x 27N reference mask collapses to N x N with a computed image index:
     g_a = -round(p_j,a - p_i,a)  in {-1,0,1}
     w_a = (g_a + p_j,a) - p_i,a      (fp32, reference-exact op order)
     hit = ((wx^2 + wy^2) + wz^2) <= 0.09f  and  j != i
     key = ((13 + gx + 3 gy + 9 gz) * 2048 + j)   (< 2^16, exact)
  argwhere packing == ascending-key order == take 256 smallest keys sorted.
  Selection runs as an oblivious bitonic top-256 on uint16 keys (sentinel
  0xFFFF): sort eight 256-chunks ascending, then 3 rounds of reversed-read
  prune-merges (8->4->2->1). Phase-1 arithmetic is distributed across
  GPSIMD/ACT/DVE (all bitwise-IEEE-identical, probe-verified) so the DVE
  mostly runs the sort network.
Host: shard/replicate inputs, decode keys -> neighbours/cell_indices, max
of per-row hit counts -> actual_max. jnp.take(idx=-1) wraps, so invalid
cell_indices slots are shifts[26] = (1,1,1).
"""
import sys

if '/opt/trn_rl_repo' not in sys.path:
    sys.path.insert(0, '/opt/trn_rl_repo')

import numpy as np

N = 2048
K = 256
CHUNK = 256
NCORES = 8
ROWS_PER_CORE = N // NCORES  # 256
NTILES = ROWS_PER_CORE // 128  # 2
SENT = 0xFFFF  # uint16 sentinel (> max key 55295)
C_BASE = 13 * 2048  # 26624
THR = 0.3 * 0.3  # fp32-converts to 0.090000004 like the jax reference

_cached = {}


def _build_program():
    import concourse.bacc as bacc
    import concourse.mybir as mybir
    from concourse.tile import TileContext

    f32 = mybir.dt.float32
    i32 = mybir.dt.int32
    u16 = mybir.dt.uint16
    u32 = mybir.dt.uint32
    Alu = mybir.AluOpType
    Act = mybir.ActivationFunctionType

    nc = bacc.Bacc("TRN2", target_bir_lowering=False)

    pjb_d = nc.dram_tensor("pjb", [3, N], f32, kind="ExternalInput")
    pit_d = nc.dram_tensor("pit", [NTILES, 128, 3], f32, kind="ExternalInput")
    ig_d = nc.dram_tensor("ig", [NTILES, 128, 1], f32, kind="ExternalInput")
    keys_d = nc.dram_tensor("keys", [NTILES, 128, K], u16, kind="ExternalOutput")
    cnt_d = nc.dram_tensor("cnt", [NTILES, 128, 1], f32, kind="ExternalOutput")

    with TileContext(nc) as tc:
        with tc.tile_pool(name="main", bufs=1) as pool:
            big = [128, N]
            pj = [pool.tile(big, f32, name=f"pj{a}", tag=f"pj{a}") for a in range(3)]
            iota_i = pool.tile(big, i32, tag="iotai")
            iota_f = pool.tile(big, f32, tag="iotaf")

            for a in range(3):
                nc.sync.dma_start(
                    out=pj[a][:],
                    in_=pjb_d[a:a + 1, :].partition_broadcast(128).squeeze(1),
                )
            nc.gpsimd.iota(iota_i[:], pattern=[[1, N]], base=C_BASE,
                           channel_multiplier=0)
            nc.vector.tensor_copy(out=iota_f[:], in_=iota_i[:])

            for t in range(NTILES):
                pit_s = pool.tile([128, 3], f32, tag=f"pit{t}")
                ig_s = pool.tile([128, 1], f32, tag=f"ig{t}")
                hi_s = pool.tile([128, 3], f32, tag=f"hi{t}")
                lo_s = pool.tile([128, 3], f32, tag=f"lo{t}")
                npi_s = pool.tile([128, 3], f32, tag=f"npi{t}")
                cnt_s = pool.tile([128, 1], f32, tag=f"cnt{t}")
                nc.sync.dma_start(out=pit_s[:], in_=pit_d[t])
                nc.sync.dma_start(out=ig_s[:], in_=ig_d[t])
                nc.vector.tensor_scalar(out=hi_s[:], in0=pit_s[:], scalar1=0.5,
                                        scalar2=None, op0=Alu.add)
                nc.vector.tensor_scalar(out=lo_s[:], in0=pit_s[:], scalar1=-0.5,
                                        scalar2=None, op0=Alu.add)
                nc.vector.tensor_scalar(out=npi_s[:], in0=pit_s[:], scalar1=-1.0,
                                        scalar2=None, op0=Alu.mult)

                s = [pool.tile(big, f32, name=f"s{a}", tag=f"s{a}") for a in range(3)]
                w = [pool.tile(big, f32, name=f"w{a}", tag=f"w{a}") for a in range(3)]
                t1 = pool.tile(big, f32, tag="t1")
                t2 = pool.tile(big, f32, tag="t2")
                d2 = pool.tile(big, f32, tag="d2")
                hitm = pool.tile(big, f32, tag="hitm")
                noteq = pool.tile(big, f32, tag="noteq")
                hitu = pool.tile(big, u32, tag=f"hitu{t}")
                k16 = pool.tile(big, u16, tag=f"k16{t}")

                for a in range(3):
                    # g = (pj < pi-0.5) - (pj > pi+0.5)   [GPSIMD, exact preds]
                    cma = pool.tile(big, f32, name=f"cma{a}", tag="cma")
                    cmb = pool.tile(big, f32, name=f"cmb{a}", tag="cmb")
                    nc.gpsimd.tensor_scalar(out=cma[:], in0=pj[a][:],
                                            scalar1=lo_s[:, a:a + 1],
                                            scalar2=None, op0=Alu.is_lt)
                    nc.gpsimd.tensor_scalar(out=cmb[:], in0=pj[a][:],
                                            scalar1=hi_s[:, a:a + 1],
                                            scalar2=None, op0=Alu.is_gt)
                    nc.gpsimd.tensor_tensor(out=s[a][:], in0=cma[:], in1=cmb[:],
                                            op=Alu.subtract)
                    # u = g + pj [GPSIMD add, IEEE-exact]
                    nc.gpsimd.tensor_tensor(out=w[a][:], in0=s[a][:],
                                            in1=pj[a][:], op=Alu.add)
                    # w = u - pi  [ACT FMA u*1 + (-pi), probe-exact]
                    nc.scalar.activation(w[a][:], w[a][:], Act.Identity,
                                         bias=npi_s[:, a:a + 1], scale=1.0)
                    # sq = w^2  [ACT Square, probe-exact]
                    nc.scalar.activation(w[a][:], w[a][:], Act.Square)

                # d2 = (sq0 + sq1) + sq2   [GPSIMD adds, exact]
                nc.gpsimd.tensor_tensor(out=t1[:], in0=w[0][:], in1=w[1][:],
                                        op=Alu.add)
                nc.gpsimd.tensor_tensor(out=d2[:], in0=t1[:], in1=w[2][:],
                                        op=Alu.add)
                # hit = d2 <= THR  [GPSIMD pred]
                nc.gpsimd.tensor_scalar(out=t1[:], in0=d2[:], scalar1=THR,
                                        scalar2=None, op0=Alu.is_le)
                # self-exclusion  [DVE not_equal]
                nc.vector.tensor_scalar(out=noteq[:], in0=iota_f[:],
                                        scalar1=ig_s[:], scalar2=None,
                                        op0=Alu.not_equal)
                nc.gpsimd.tensor_tensor(out=hitm[:], in0=t1[:], in1=noteq[:],
                                        op=Alu.mult)
                # mask cast to u32 + per-row count in one ACT op
                nc.scalar.activation(hitu[:], hitm[:], Act.Copy,
                                     accum_out=cnt_s[:])
                nc.sync.dma_start(out=cnt_d[t], in_=cnt_s[:])

                # key = ((gx + 3 gy + 9 gz) * 2048) + (26624 + j)
                nc.gpsimd.tensor_scalar(out=t1[:], in0=s[1][:], scalar1=3.0,
                                        scalar2=None, op0=Alu.mult)
                nc.gpsimd.tensor_tensor(out=t2[:], in0=t1[:], in1=s[0][:],
                                        op=Alu.add)
                nc.gpsimd.tensor_scalar(out=t1[:], in0=s[2][:], scalar1=9.0,
                                        scalar2=None, op0=Alu.mult)
                nc.gpsimd.tensor_tensor(out=t2[:], in0=t2[:], in1=t1[:],
                                        op=Alu.add)
                # k3 = t2*2048 + iota  [DVE stt]
                nc.vector.scalar_tensor_tensor(out=d2[:], in0=t2[:],
                                               scalar=2048.0, in1=iota_f[:],
                                               op0=Alu.mult, op1=Alu.add)
                # cast fp32 -> uint16 keys [ACT copy]
                nc.scalar.activation(k16[:], d2[:], Act.Copy)

                A = pool.tile(big, u16, tag=f"A{t}")
                B = pool.tile(big, u16, tag=f"B{t}")
                nc.gpsimd.memset(A[:], SENT)
                nc.vector.copy_predicated(A[:], hitu[:], k16[:])

                # ---- phase 2: bitonic top-256 on uint16 keys [DVE] ----
                cur, other = A, B

                def substage(lo_in, hi_in, lo_out, hi_out):
                    nonlocal cur, other
                    nc.vector.tensor_tensor(out=lo_out, in0=lo_in, in1=hi_in,
                                            op=Alu.min)
                    nc.vector.tensor_tensor(out=hi_out, in0=lo_in, in1=hi_in,
                                            op=Alu.max)
                    cur, other = other, cur

                def dist_substage(width, d):
                    r_in = cur[:, :width].rearrange("p (b r) -> p b r", r=2 * d)
                    r_out = other[:, :width].rearrange("p (b r) -> p b r", r=2 * d)
                    substage(r_in[:, :, 0:d], r_in[:, :, d:2 * d],
                             r_out[:, :, 0:d], r_out[:, :, d:2 * d])

                # stage A: sort each 256-chunk ascending
                for mexp in range(8):
                    m = 1 << mexp
                    r_in = cur[:].rearrange("p (b r) -> p b r", r=2 * m)
                    r_out = other[:].rearrange("p (b r) -> p b r", r=2 * m)
                    substage(r_in[:, :, 0:m], r_in[:, :, m:2 * m][:, :, ::-1],
                             r_out[:, :, 0:m], r_out[:, :, m:2 * m][:, :, ::-1])
                    d = m // 2
                    while d >= 1:
                        dist_substage(N, d)
                        d //= 2

                # stage B: prune-merges 8 -> 4 -> 2 -> 1 lists of 256
                width = N
                while width > CHUNK:
                    half = width // 2
                    r_in = cur[:, :width].rearrange("p (l r) -> p l r",
                                                    r=2 * CHUNK)
                    r_out = other[:, :half].rearrange("p (l r) -> p l r",
                                                      r=CHUNK)
                    nc.vector.tensor_tensor(
                        out=r_out[:],
                        in0=r_in[:, :, 0:CHUNK],
                        in1=r_in[:, :, CHUNK:2 * CHUNK][:, :, ::-1],
                        op=Alu.min)
                    cur, other = other, cur
                    d = CHUNK // 2
                    while d >= 1:
                        dist_substage(half, d)
                        d //= 2
                    width = half

                nc.sync.dma_start(out=keys_d[t], in_=cur[:, :K])

    nc.compile()
    return nc


def _get_program():
    if "nc" not in _cached:
        _cached["nc"] = _build_program()
    return _cached["nc"]


def _make_in_maps(pos):
    pjb = np.ascontiguousarray(pos.T)  # [3, N]
    in_maps = []
    for cr in range(NCORES):
        rows0 = cr * ROWS_PER_CORE
        pit = pos[rows0: rows0 + ROWS_PER_CORE].reshape(NTILES, 128, 3)
        ig = (C_BASE + rows0 + np.arange(ROWS_PER_CORE, dtype=np.float32)
              ).reshape(NTILES, 128, 1).astype(np.float32)
        in_maps.append({
            "pjb": pjb,
            "pit": np.ascontiguousarray(pit),
            "ig": np.ascontiguousarray(ig),
        })
    return in_maps


def kernel(positions, cell, max_neighbours):
    from concourse.bass_utils import run_bass_kernel_spmd

    pos = np.asarray(positions, dtype=np.float32)
    assert pos.shape == (N, 3)
    k = int(max_neighbours)
    assert k == K, f"kernel hardcodes max_neighbours=256, got {k}"

    nc = _get_program()
    res = run_bass_kernel_spmd(nc, _make_in_maps(pos),
                               core_ids=list(range(NCORES)))

    keys = np.concatenate(
        [r["keys"].reshape(ROWS_PER_CORE, K) for r in res.results], axis=0)
    counts = np.concatenate(
        [r["cnt"].reshape(ROWS_PER_CORE) for r in res.results], axis=0)

    valid = keys != SENT
    ki = keys.astype(np.int64)
    j = ki & (N - 1)
    c = ki >> 11
    neighbours = np.where(valid, j, -1).astype(np.int32)
    sx = c % 3 - 1
    sy = (c // 3) % 3 - 1
    sz = c // 9 - 1
    cells = np.stack([sx, sy, sz], axis=-1)
    cells = np.where(valid[..., None], cells, 1).astype(np.int32)
    actual_max = np.int32(counts.max())
    return neighbours, cells, actual_max


# revision 5
# speedup vs baseline: 2.1684x; 2.1684x over previous
"""Periodic-boundary fixed-capacity neighbour list on 8 trn2 NeuronCores.

Device algorithm (per core, 256 rows as 2 partition-tiles):
  For unit cell + cutoff 0.3, a pair (i, j) is within cutoff for at most ONE
  of the 27 periodic images, and per axis at most TWO image shifts are ever
  feasible for a given centre row ({0,+1} if p_i>0.5 else {-1,0}).  So the
  N x 27N reference mask collapses to N x N with a 3-bit reduced cell code:
     g_a = -round(p_j,a - p_i,a)            (via two compares)
     w_a = (g_a + p_j,a) - p_i,a           (fp32, reference-exact op order)
     hit = ((wx^2+wy^2)+wz^2) <= 0.09f  and  j != i
     b_a = [u_a >= flip2_a],  flip2_a = [p_i,a > 0.5]   (order bit per axis)
     key = (bz*4+by*2+bx)*2048 + j + 8192  in [8192, 24576) ; miss -> 32512
  Ascending-key order == the reference argwhere packing order.  Keys are
  cast to uint16 and REINTERPRETED as bf16 (positive-float bit patterns
  sort like integers), so the bitonic top-256 network runs mostly in the
  DVE's 2x 16-bit mode.  Phase-1 arithmetic is spread over GPSIMD (tensor-
  tensor only) and ACT (FMA/Square, bitwise-IEEE verified) so the DVE
  mostly runs the sort network.
Host: shard/replicate inputs, decode keys -> neighbours/cell_indices
(b-bit + host-side flip rule -> shift vector), max of per-row hit counts ->
actual_max.  jnp.take(idx=-1) wraps: invalid cell slots = shifts[26] =
(1,1,1).
"""
import sys

if '/opt/trn_rl_repo' not in sys.path:
    sys.path.insert(0, '/opt/trn_rl_repo')

import numpy as np

N = 2048
K = 256
CHUNK = 256
NCORES = 8
ROWS_PER_CORE = N // NCORES  # 256
NTILES = ROWS_PER_CORE // 128  # 2
OFF = 8192
SENT = 32512  # 0x7F00 as uint16; huge finite positive as bf16
THR = 0.3 * 0.3  # fp32-converts to 0.090000004 like the jax reference

_cached = {}


def _build_program():
    import concourse.bacc as bacc
    import concourse.mybir as mybir
    from concourse.tile import TileContext

    f32 = mybir.dt.float32
    i32 = mybir.dt.int32
    u16 = mybir.dt.uint16
    bf16 = mybir.dt.bfloat16
    Alu = mybir.AluOpType
    Act = mybir.ActivationFunctionType
    Ax = mybir.AxisListType

    nc = bacc.Bacc("TRN2", target_bir_lowering=False)

    pjb_d = nc.dram_tensor("pjb", [3, N], f32, kind="ExternalInput")
    pit_d = nc.dram_tensor("pit", [NTILES, 128, 3], f32, kind="ExternalInput")
    ig_d = nc.dram_tensor("ig", [NTILES, 128, 1], f32, kind="ExternalInput")
    fl_d = nc.dram_tensor("fl", [NTILES, 128, 3], f32, kind="ExternalInput")
    keys_d = nc.dram_tensor("keys", [NTILES, 128, K], u16, kind="ExternalOutput")
    cnt_d = nc.dram_tensor("cnt", [NTILES, 128, 1], f32, kind="ExternalOutput")

    with TileContext(nc) as tc:
        with tc.tile_pool(name="main", bufs=1) as pool:
            big = [128, N]
            pj = [pool.tile(big, f32, name=f"pj{a}", tag=f"pj{a}") for a in range(3)]
            iota_i = pool.tile(big, i32, tag="iotai")
            iota_f = pool.tile(big, f32, tag="iotaf")

            for a in range(3):
                nc.sync.dma_start(
                    out=pj[a][:],
                    in_=pjb_d[a:a + 1, :].partition_broadcast(128).squeeze(1),
                )
            nc.gpsimd.iota(iota_i[:], pattern=[[1, N]], base=OFF,
                           channel_multiplier=0)
            nc.vector.tensor_copy(out=iota_f[:], in_=iota_i[:])

            for t in range(NTILES):
                pit_s = pool.tile([128, 3], f32, tag=f"pit{t}")
                ig_s = pool.tile([128, 1], f32, tag=f"ig{t}")
                fl_s = pool.tile([128, 3], f32, tag=f"fl{t}")
                hi_s = pool.tile([128, 3], f32, tag=f"hi{t}")
                lo_s = pool.tile([128, 3], f32, tag=f"lo{t}")
                npi_s = pool.tile([128, 3], f32, tag=f"npi{t}")
                cnt_s = pool.tile([128, 1], f32, tag=f"cnt{t}")
                nc.sync.dma_start(out=pit_s[:], in_=pit_d[t])
                nc.sync.dma_start(out=ig_s[:], in_=ig_d[t])
                nc.sync.dma_start(out=fl_s[:], in_=fl_d[t])
                nc.vector.tensor_scalar(out=hi_s[:], in0=pit_s[:], scalar1=0.5,
                                        scalar2=None, op0=Alu.add)
                nc.vector.tensor_scalar(out=lo_s[:], in0=pit_s[:], scalar1=-0.5,
                                        scalar2=None, op0=Alu.add)
                nc.vector.tensor_scalar(out=npi_s[:], in0=pit_s[:], scalar1=-1.0,
                                        scalar2=None, op0=Alu.mult)

                s = [pool.tile(big, f32, name=f"s{a}", tag=f"s{a}") for a in range(3)]
                u = [pool.tile(big, f32, name=f"u{a}", tag=f"u{a}") for a in range(3)]
                bt = [pool.tile(big, f32, name=f"bt{a}", tag=f"bt{a}") for a in range(3)]
                t1 = pool.tile(big, f32, tag="t1")
                t2 = pool.tile(big, f32, tag="t2")
                d2 = pool.tile(big, f32, tag="d2")
                hitm = pool.tile(big, f32, tag="hitm")
                noteq = pool.tile(big, f32, tag="noteq")

                for a in range(3):
                    cma = pool.tile(big, f32, name=f"cma{a}", tag="cma")
                    cmb = pool.tile(big, f32, name=f"cmb{a}", tag="cmb")
                    # g = (pj < pi-0.5) - (pj > pi+0.5)   [DVE preds, 2x]
                    nc.vector.tensor_scalar(out=cma[:], in0=pj[a][:],
                                            scalar1=lo_s[:, a:a + 1],
                                            scalar2=None, op0=Alu.is_lt)
                    nc.vector.tensor_scalar(out=cmb[:], in0=pj[a][:],
                                            scalar1=hi_s[:, a:a + 1],
                                            scalar2=None, op0=Alu.is_gt)
                    nc.gpsimd.tensor_tensor(out=s[a][:], in0=cma[:], in1=cmb[:],
                                            op=Alu.subtract)
                    # u = g + pj [GPSIMD add, IEEE-exact]
                    nc.gpsimd.tensor_tensor(out=u[a][:], in0=s[a][:],
                                            in1=pj[a][:], op=Alu.add)
                    # order bit * axis weight: (u >= flip2) * 2^(11+a) [DVE 2x]
                    nc.vector.tensor_scalar(out=bt[a][:], in0=u[a][:],
                                            scalar1=fl_s[:, a:a + 1],
                                            scalar2=float(2048 << a),
                                            op0=Alu.is_ge, op1=Alu.mult)
                    # w = u - pi  [ACT FMA u*1 + (-pi), probe-exact]
                    nc.scalar.activation(u[a][:], u[a][:], Act.Identity,
                                         bias=npi_s[:, a:a + 1], scale=1.0)
                    # sq = w^2  [ACT Square, probe-exact]
                    nc.scalar.activation(u[a][:], u[a][:], Act.Square)

                # d2 = (sq0 + sq1) + sq2   [GPSIMD adds, exact]
                nc.gpsimd.tensor_tensor(out=t1[:], in0=u[0][:], in1=u[1][:],
                                        op=Alu.add)
                nc.gpsimd.tensor_tensor(out=d2[:], in0=t1[:], in1=u[2][:],
                                        op=Alu.add)
                # hit = d2 <= THR [DVE 2x];  self-exclusion [DVE]
                nc.vector.tensor_scalar(out=t2[:], in0=d2[:], scalar1=THR,
                                        scalar2=None, op0=Alu.is_le)
                nc.vector.tensor_scalar(out=noteq[:], in0=iota_f[:],
                                        scalar1=ig_s[:], scalar2=None,
                                        op0=Alu.not_equal)
                nc.gpsimd.tensor_tensor(out=hitm[:], in0=t2[:], in1=noteq[:],
                                        op=Alu.mult)
                nc.vector.tensor_reduce(out=cnt_s[:], in_=hitm[:], axis=Ax.X,
                                        op=Alu.add)
                nc.sync.dma_start(out=cnt_d[t], in_=cnt_s[:])

                # key' = bx' + by' + bz' + (8192 + j)    [GPSIMD adds]
                nc.gpsimd.tensor_tensor(out=t1[:], in0=bt[0][:], in1=bt[1][:],
                                        op=Alu.add)
                nc.gpsimd.tensor_tensor(out=t2[:], in0=t1[:], in1=bt[2][:],
                                        op=Alu.add)
                nc.gpsimd.tensor_tensor(out=t1[:], in0=t2[:], in1=iota_f[:],
                                        op=Alu.add)
                # sentinel select: kf = (key'-SENT)*hitm + SENT  [DVE, exact]
                nc.vector.scalar_tensor_tensor(out=t2[:], in0=t1[:],
                                               scalar=float(SENT),
                                               in1=hitm[:], op0=Alu.subtract,
                                               op1=Alu.mult)
                nc.vector.tensor_scalar(out=d2[:], in0=t2[:],
                                        scalar1=float(SENT), scalar2=None,
                                        op0=Alu.add)

                A = pool.tile(big, bf16, name=f"A{t}", tag=f"A{t}")
                B = pool.tile(big, bf16, name=f"B{t}", tag=f"B{t}")
                # cast fp32 int-valued -> uint16 bit patterns, into A
                nc.scalar.activation(A[:].bitcast(u16), d2[:], Act.Copy)

                # ---- phase 2: bitonic top-256 on bf16 bit patterns [DVE] ----
                cur, other = A, B

                def swap():
                    nonlocal cur, other
                    cur, other = other, cur

                def cx(lo_in, hi_in, lo_out, hi_out, asc):
                    nc.vector.tensor_tensor(out=lo_out, in0=lo_in, in1=hi_in,
                                            op=Alu.min if asc else Alu.max)
                    nc.vector.tensor_tensor(out=hi_out, in0=lo_in, in1=hi_in,
                                            op=Alu.max if asc else Alu.min)

                # stage A: sort each 256-chunk ascending, alternating dirs
                for kexp in range(1, 9):
                    kk = 1 << kexp  # 2..256
                    j = kk // 2
                    while j >= 1:
                        if kk < 256:
                            # [p, bk, t(2), bj, r(2j)] ; t=0 asc, t=1 desc
                            vi = cur[:].rearrange(
                                "p (bk t bj r) -> p bk t bj r",
                                t=2, bj=kk // (2 * j), r=2 * j)
                            vo = other[:].rearrange(
                                "p (bk t bj r) -> p bk t bj r",
                                t=2, bj=kk // (2 * j), r=2 * j)
                            cx(vi[:, :, 0, :, 0:j], vi[:, :, 0, :, j:2 * j],
                               vo[:, :, 0, :, 0:j], vo[:, :, 0, :, j:2 * j],
                               asc=True)
                            cx(vi[:, :, 1, :, 0:j], vi[:, :, 1, :, j:2 * j],
                               vo[:, :, 1, :, 0:j], vo[:, :, 1, :, j:2 * j],
                               asc=False)
                        else:
                            vi = cur[:].rearrange("p (b r) -> p b r", r=2 * j)
                            vo = other[:].rearrange("p (b r) -> p b r", r=2 * j)
                            cx(vi[:, :, 0:j], vi[:, :, j:2 * j],
                               vo[:, :, 0:j], vo[:, :, j:2 * j], asc=True)
                        swap()
                        j //= 2

                # stage B: prune-merges 8 -> 4 -> 2 -> 1 lists of 256
                width = N
                while width > CHUNK:
                    half = width // 2
                    vi = cur[:, :width].rearrange("p (l r) -> p l r",
                                                  r=2 * CHUNK)
                    vo = other[:, :half].rearrange("p (l r) -> p l r", r=CHUNK)
                    nc.vector.tensor_tensor(
                        out=vo[:],
                        in0=vi[:, :, 0:CHUNK],
                        in1=vi[:, :, CHUNK:2 * CHUNK][:, :, ::-1],
                        op=Alu.min)
                    swap()
                    d = CHUNK // 2
                    while d >= 1:
                        vi = cur[:, :half].rearrange("p (b r) -> p b r", r=2 * d)
                        vo = other[:, :half].rearrange("p (b r) -> p b r", r=2 * d)
                        cx(vi[:, :, 0:d], vi[:, :, d:2 * d],
                           vo[:, :, 0:d], vo[:, :, d:2 * d], asc=True)
                        swap()
                        d //= 2
                    width = half

                nc.sync.dma_start(out=keys_d[t], in_=cur[:, :K].bitcast(u16))

    nc.compile()
    return nc


def _get_program():
    if "nc" not in _cached:
        _cached["nc"] = _build_program()
    return _cached["nc"]


def _make_in_maps(pos):
    pjb = np.ascontiguousarray(pos.T)  # [3, N]
    in_maps = []
    for cr in range(NCORES):
        rows0 = cr * ROWS_PER_CORE
        pit = pos[rows0: rows0 + ROWS_PER_CORE].reshape(NTILES, 128, 3)
        ig = (OFF + rows0 + np.arange(ROWS_PER_CORE, dtype=np.float32)
              ).reshape(NTILES, 128, 1).astype(np.float32)
        fl = (pit > 0.5).astype(np.float32)
        in_maps.append({
            "pjb": pjb,
            "pit": np.ascontiguousarray(pit),
            "ig": np.ascontiguousarray(ig),
            "fl": np.ascontiguousarray(fl),
        })
    return in_maps


def kernel(positions, cell, max_neighbours):
    from concourse.bass_utils import run_bass_kernel_spmd

    pos = np.asarray(positions, dtype=np.float32)
    assert pos.shape == (N, 3)
    k = int(max_neighbours)
    assert k == K, f"kernel hardcodes max_neighbours=256, got {k}"

    nc = _get_program()
    res = run_bass_kernel_spmd(nc, _make_in_maps(pos),
                               core_ids=list(range(NCORES)))

    keys = np.concatenate(
        [r["keys"].reshape(ROWS_PER_CORE, K) for r in res.results], axis=0)
    counts = np.concatenate(
        [r["cnt"].reshape(ROWS_PER_CORE) for r in res.results], axis=0)

    raw = keys.astype(np.int64)
    valid = raw != SENT
    key = raw - OFF
    j = key & (N - 1)
    cp = key >> 11
    bx = cp & 1
    by = (cp >> 1) & 1
    bz = cp >> 2
    f = pos > 0.5  # same rule as device flip2
    g = np.empty((N, K, 3), np.int64)
    for a, bbit in enumerate((bx, by, bz)):
        fa = f[:, a][:, None]
        g[:, :, a] = np.where(fa, np.where(bbit > 0, 1, 0),
                              np.where(bbit > 0, 0, -1))
    neighbours = np.where(valid, j, -1).astype(np.int32)
    cells = np.where(valid[..., None], g, 1).astype(np.int32)
    actual_max = np.int32(counts.max())
    return neighbours, cells, actual_max


# revision 6
# speedup vs baseline: 2.7607x; 1.2731x over previous
"""Periodic-boundary fixed-capacity neighbour list on 8 trn2 NeuronCores.

Device algorithm (per core, 256 rows as 2 partition-tiles):
  For unit cell + cutoff 0.3, a pair (i, j) is within cutoff for at most ONE
  of the 27 periodic images, and per axis at most TWO image shifts are ever
  feasible for a given centre row ({0,+1} if p_i>0.5 else {-1,0}).  So the
  N x 27N reference mask collapses to N x N with a 3-bit reduced cell code:
     v_a = p_j,a - p_i,a          (ACT FMA; compare vs +-0.5 immediates)
     g_a = [v<-0.5] - [v>0.5]     (threshold-epsilon differences are no-hit
                                   safe: |w| would be ~0.5 >> 0.3)
     u_a = g_a + p_j,a ;  w_a = u_a - p_i,a    (fp32, reference-exact order)
     hit = ((wx^2+wy^2)+wz^2) <= 0.09f  and  j != i
     b_a = [u_a - flip2_a >= 0]   (exact: Sterbenz),  flip2_a = [p_i,a>0.5]
     key = (bz*4+by*2+bx)*2048 + j + 8192  in [8192, 24576) ; miss -> 32512
  Ascending-key order == the reference argwhere packing order.  Keys are
  cast to uint16 and REINTERPRETED as bf16 (positive-float bit patterns
  sort like integers), so the bitonic top-256 (sort eight 256-chunks, then
  reversed-read prune-merges 8->4->2->1) runs mostly in the DVE 2x 16-bit
  mode.  Phase-1 is spread over ACT (FMA/Square, bitwise-IEEE verified),
  GPSIMD (tensor-tensor add/sub/mult only) and DVE immediate-scalar
  compares, so the DVE mostly runs the sort network.
Host: shard/replicate inputs, decode keys -> neighbours/cell_indices
(b-bit + flip rule -> shift vector), max of per-row hit counts ->
actual_max.  jnp.take(idx=-1) wraps: invalid cell slots = shifts[26] =
(1,1,1).
"""
import sys

if '/opt/trn_rl_repo' not in sys.path:
    sys.path.insert(0, '/opt/trn_rl_repo')

import numpy as np

N = 2048
K = 256
CHUNK = 256
NCORES = 8
ROWS_PER_CORE = N // NCORES  # 256
NTILES = ROWS_PER_CORE // 128  # 2
OFF = 8192
SENT = 32512  # 0x7F00 as uint16; huge finite positive as bf16
THR = 0.3 * 0.3  # fp32-converts to 0.090000004 like the jax reference

_cached = {}


def _build_program():
    import concourse.bacc as bacc
    import concourse.mybir as mybir
    from concourse.tile import TileContext

    f32 = mybir.dt.float32
    i32 = mybir.dt.int32
    u16 = mybir.dt.uint16
    bf16 = mybir.dt.bfloat16
    Alu = mybir.AluOpType
    Act = mybir.ActivationFunctionType
    Ax = mybir.AxisListType

    nc = bacc.Bacc("TRN2", target_bir_lowering=False)

    pjb_d = nc.dram_tensor("pjb", [3, N], f32, kind="ExternalInput")
    pit_d = nc.dram_tensor("pit", [NTILES, 128, 3], f32, kind="ExternalInput")
    ig_d = nc.dram_tensor("ig", [NTILES, 128, 1], f32, kind="ExternalInput")
    fl_d = nc.dram_tensor("fl", [NTILES, 128, 3], f32, kind="ExternalInput")
    keys_d = nc.dram_tensor("keys", [NTILES, 128, K], u16, kind="ExternalOutput")
    cnt_d = nc.dram_tensor("cnt", [NTILES, 128, 1], f32, kind="ExternalOutput")

    with TileContext(nc) as tc:
        with tc.tile_pool(name="main", bufs=1) as pool:
            big = [128, N]
            pj = [pool.tile(big, f32, name=f"pj{a}", tag=f"pj{a}") for a in range(3)]
            iota_i = pool.tile(big, i32, tag="iotai")
            iota_f = pool.tile(big, f32, tag="iotaf")

            for a in range(3):
                nc.sync.dma_start(
                    out=pj[a][:],
                    in_=pjb_d[a:a + 1, :].partition_broadcast(128).squeeze(1),
                )
            nc.gpsimd.iota(iota_i[:], pattern=[[1, N]], base=OFF,
                           channel_multiplier=0)
            nc.vector.tensor_copy(out=iota_f[:], in_=iota_i[:])

            for t in range(NTILES):
                pit_s = pool.tile([128, 3], f32, tag=f"pit{t}")
                nig_s = pool.tile([128, 1], f32, tag=f"nig{t}")
                nfl_s = pool.tile([128, 3], f32, tag=f"nfl{t}")
                npi_s = pool.tile([128, 3], f32, tag=f"npi{t}")
                cnt_s = pool.tile([128, 1], f32, tag=f"cnt{t}")
                nc.sync.dma_start(out=pit_s[:], in_=pit_d[t])
                ig_s = pool.tile([128, 1], f32, tag=f"ig{t}")
                fl_s = pool.tile([128, 3], f32, tag=f"fl{t}")
                nc.sync.dma_start(out=ig_s[:], in_=ig_d[t])
                nc.sync.dma_start(out=fl_s[:], in_=fl_d[t])
                nc.vector.tensor_scalar(out=npi_s[:], in0=pit_s[:], scalar1=-1.0,
                                        scalar2=None, op0=Alu.mult)
                nc.vector.tensor_scalar(out=nig_s[:], in0=ig_s[:], scalar1=-1.0,
                                        scalar2=None, op0=Alu.mult)
                nc.vector.tensor_scalar(out=nfl_s[:], in0=fl_s[:], scalar1=-1.0,
                                        scalar2=None, op0=Alu.mult)

                u = [pool.tile(big, f32, name=f"u{a}", tag=f"u{a}") for a in range(3)]
                bt = [pool.tile(big, f32, name=f"bt{a}", tag=f"bt{a}") for a in range(3)]
                t1 = pool.tile(big, f32, tag="t1")
                t2 = pool.tile(big, f32, tag="t2")
                d2 = pool.tile(big, f32, tag="d2")
                hitm = pool.tile(big, f32, tag="hitm")
                noteq = pool.tile(big, f32, tag="noteq")

                for a in range(3):
                    va = pool.tile(big, f32, name=f"va{a}", tag="va")
                    cma = pool.tile(big, f32, name=f"cma{a}", tag="cma")
                    cmb = pool.tile(big, f32, name=f"cmb{a}", tag="cmb")
                    uba = pool.tile(big, f32, name=f"uba{a}", tag="uba")
                    # v = pj - pi  [ACT FMA, bit-exact]
                    nc.scalar.activation(va[:], pj[a][:], Act.Identity,
                                         bias=npi_s[:, a:a + 1], scale=1.0)
                    # g parts via immediate compares [DVE 2x]
                    nc.vector.tensor_scalar(out=cma[:], in0=va[:],
                                            scalar1=-0.5, scalar2=None,
                                            op0=Alu.is_lt)
                    nc.vector.tensor_scalar(out=cmb[:], in0=va[:],
                                            scalar1=0.5, scalar2=None,
                                            op0=Alu.is_gt)
                    # g = cma - cmb ; u = g + pj   [GPSIMD]
                    nc.gpsimd.tensor_tensor(out=t1[:], in0=cma[:], in1=cmb[:],
                                            op=Alu.subtract)
                    nc.gpsimd.tensor_tensor(out=u[a][:], in0=t1[:],
                                            in1=pj[a][:], op=Alu.add)
                    # order bit: (u - flip2 >= 0) * 2^(11+a)
                    nc.scalar.activation(uba[:], u[a][:], Act.Identity,
                                         bias=nfl_s[:, a:a + 1], scale=1.0)
                    nc.vector.tensor_scalar(out=bt[a][:], in0=uba[:],
                                            scalar1=0.0,
                                            scalar2=float(2048 << a),
                                            op0=Alu.is_ge, op1=Alu.mult)
                    # w = u - pi ; sq = w^2  [ACT, bit-exact, in-place]
                    nc.scalar.activation(u[a][:], u[a][:], Act.Identity,
                                         bias=npi_s[:, a:a + 1], scale=1.0)
                    nc.scalar.activation(u[a][:], u[a][:], Act.Square)

                # d2 = (sq0 + sq1) + sq2   [DVE]
                nc.vector.tensor_tensor(out=t2[:], in0=u[0][:], in1=u[1][:],
                                        op=Alu.add)
                nc.vector.tensor_tensor(out=d2[:], in0=t2[:], in1=u[2][:],
                                        op=Alu.add)
                # hit = d2 <= THR [DVE imm]
                nc.vector.tensor_scalar(out=t2[:], in0=d2[:], scalar1=THR,
                                        scalar2=None, op0=Alu.is_le)
                # self-exclusion: iota != ig via ACT diff + imm compare
                vn = pool.tile(big, f32, name=f"vn{t}", tag="va")
                nc.scalar.activation(vn[:], iota_f[:], Act.Identity,
                                     bias=nig_s[:], scale=1.0)
                nc.vector.tensor_scalar(out=noteq[:], in0=vn[:], scalar1=0.0,
                                        scalar2=None, op0=Alu.not_equal)
                nc.gpsimd.tensor_tensor(out=hitm[:], in0=t2[:], in1=noteq[:],
                                        op=Alu.mult)
                nc.vector.tensor_reduce(out=cnt_s[:], in_=hitm[:], axis=Ax.X,
                                        op=Alu.add)
                nc.sync.dma_start(out=cnt_d[t], in_=cnt_s[:])

                # key' = bx' + by' + bz' + (8192 + j)    [GPSIMD adds]
                nc.gpsimd.tensor_tensor(out=t1[:], in0=bt[0][:], in1=bt[1][:],
                                        op=Alu.add)
                nc.gpsimd.tensor_tensor(out=t2[:], in0=t1[:], in1=bt[2][:],
                                        op=Alu.add)
                nc.gpsimd.tensor_tensor(out=t1[:], in0=t2[:], in1=iota_f[:],
                                        op=Alu.add)
                # sentinel select: kf = (key'-SENT)*hitm + SENT  [DVE, exact]
                nc.vector.scalar_tensor_tensor(out=t2[:], in0=t1[:],
                                               scalar=float(SENT),
                                               in1=hitm[:], op0=Alu.subtract,
                                               op1=Alu.mult)
                nc.vector.tensor_scalar(out=t1[:], in0=t2[:],
                                        scalar1=float(SENT), scalar2=None,
                                        op0=Alu.add)

                A = pool.tile(big, bf16, name=f"A{t}", tag=f"A{t}")
                B = pool.tile(big, bf16, name=f"B{t}", tag=f"B{t}")
                # cast fp32 int-valued -> uint16 bit patterns, into A
                nc.scalar.activation(A[:].bitcast(u16), t1[:], Act.Copy)

                # ---- phase 2: bitonic top-256 on bf16 bit patterns [DVE] ----
                cur, other = A, B

                def substage(lo_in, hi_in, lo_out, hi_out):
                    nonlocal cur, other
                    nc.vector.tensor_tensor(out=lo_out, in0=lo_in, in1=hi_in,
                                            op=Alu.min)
                    nc.vector.tensor_tensor(out=hi_out, in0=lo_in, in1=hi_in,
                                            op=Alu.max)
                    cur, other = other, cur

                def dist_substage(width, d):
                    r_in = cur[:, :width].rearrange("p (b r) -> p b r", r=2 * d)
                    r_out = other[:, :width].rearrange("p (b r) -> p b r", r=2 * d)
                    substage(r_in[:, :, 0:d], r_in[:, :, d:2 * d],
                             r_out[:, :, 0:d], r_out[:, :, d:2 * d])

                # stage A: sort each 256-chunk ascending (reversed-read merges)
                for mexp in range(8):
                    m = 1 << mexp
                    r_in = cur[:].rearrange("p (b r) -> p b r", r=2 * m)
                    r_out = other[:].rearrange("p (b r) -> p b r", r=2 * m)
                    substage(r_in[:, :, 0:m], r_in[:, :, m:2 * m][:, :, ::-1],
                             r_out[:, :, 0:m], r_out[:, :, m:2 * m][:, :, ::-1])
                    d = m // 2
                    while d >= 1:
                        dist_substage(N, d)
                        d //= 2

                # stage B: prune-merges 8 -> 4 -> 2 -> 1 lists of 256
                width = N
                while width > CHUNK:
                    half = width // 2
                    r_in = cur[:, :width].rearrange("p (l r) -> p l r",
                                                    r=2 * CHUNK)
                    r_out = other[:, :half].rearrange("p (l r) -> p l r",
                                                      r=CHUNK)
                    nc.vector.tensor_tensor(
                        out=r_out[:],
                        in0=r_in[:, :, 0:CHUNK],
                        in1=r_in[:, :, CHUNK:2 * CHUNK][:, :, ::-1],
                        op=Alu.min)
                    cur, other = other, cur
                    d = CHUNK // 2
                    while d >= 1:
                        dist_substage(half, d)
                        d //= 2
                    width = half

                nc.sync.dma_start(out=keys_d[t], in_=cur[:, :K].bitcast(u16))

    nc.compile()
    return nc


def _get_program():
    if "nc" not in _cached:
        _cached["nc"] = _build_program()
    return _cached["nc"]


def _make_in_maps(pos):
    pjb = np.ascontiguousarray(pos.T)  # [3, N]
    in_maps = []
    for cr in range(NCORES):
        rows0 = cr * ROWS_PER_CORE
        pit = pos[rows0: rows0 + ROWS_PER_CORE].reshape(NTILES, 128, 3)
        ig = (OFF + rows0 + np.arange(ROWS_PER_CORE, dtype=np.float32)
              ).reshape(NTILES, 128, 1).astype(np.float32)
        fl = (pit > 0.5).astype(np.float32)
        in_maps.append({
            "pjb": pjb,
            "pit": np.ascontiguousarray(pit),
            "ig": np.ascontiguousarray(ig),
            "fl": np.ascontiguousarray(fl),
        })
    return in_maps


def kernel(positions, cell, max_neighbours):
    from concourse.bass_utils import run_bass_kernel_spmd

    pos = np.asarray(positions, dtype=np.float32)
    assert pos.shape == (N, 3)
    k = int(max_neighbours)
    assert k == K, f"kernel hardcodes max_neighbours=256, got {k}"

    nc = _get_program()
    res = run_bass_kernel_spmd(nc, _make_in_maps(pos),
                               core_ids=list(range(NCORES)))

    keys = np.concatenate(
        [r["keys"].reshape(ROWS_PER_CORE, K) for r in res.results], axis=0)
    counts = np.concatenate(
        [r["cnt"].reshape(ROWS_PER_CORE) for r in res.results], axis=0)

    raw = keys.astype(np.int64)
    valid = raw != SENT
    key = raw - OFF
    j = key & (N - 1)
    cp = key >> 11
    bx = cp & 1
    by = (cp >> 1) & 1
    bz = cp >> 2
    f = pos > 0.5  # same rule as device flip2
    g = np.empty((N, K, 3), np.int64)
    for a, bbit in enumerate((bx, by, bz)):
        fa = f[:, a][:, None]
        g[:, :, a] = np.where(fa, np.where(bbit > 0, 1, 0),
                              np.where(bbit > 0, 0, -1))
    neighbours = np.where(valid, j, -1).astype(np.int32)
    cells = np.where(valid[..., None], g, 1).astype(np.int32)
    actual_max = np.int32(counts.max())
    return neighbours, cells, actual_max
